# revision 8
# baseline (speedup 1.0000x reference)
"""Trainium2 Bass kernel for nn_BOW (bag-of-words MLP).

emb = relu(relu(relu(bow(idx) @ W1.T + b1) @ W2.T + b2) @ W3.T + b3)

Design (8 NeuronCores):

fc1 is sharded over the vocab axis: core c owns rows [c*6400, (c+1)*6400) of
W1T (50000 padded to 51200).  W1 is stored as an fp8e4m3 two-term residual
split A + B (A = e4m3(S*W1), B = e4m3(S*W1 - A), shared scale S = 2^10),
which matches bf16 end-to-end accuracy while letting fc1 run in DoubleRow
fp8 perf mode: each matmul contracts TWO 128-lane vocab buckets per pass
(adjacent bucket pairs of the same table), so the dense fc1 runs at 2x bf16
MAC throughput while streaming the same 12.8 MB/core of weights.

Histogram without scatter: tokens are host-routed to (core, vocab-bucket of
128, batch-row-half) slots.  For each 128-token tile, one TensorE matmul
R^T @ M accumulates exact counts into PSUM, where R (token -> vocab lane)
and M (token -> row) are one-hot matrices built on DVE by iota-compare from
tiny per-slot id vectors (rv, rw) -- no one-hot streaming from HBM.  The
PSUM counts are copied to fp8 bowT on the Scalar engine (counts <= 15 are
exact in e4m3), feeding stage-2 DoubleRow matmuls as lhsT bucket pairs.

Partial h1 [256, 1024] (descaled by 1/S during the PSUM->bf16 copy) is
exchanged with a single AllToAll and summed on TensorE with a
stacked-identity matmul whose output is ALREADY transposed (features on
partitions), so the bf16 fc2/fc3 tail needs no PE transposes; biases fold in
via per-partition activation bias or a ones-vector matmul.  A tiny
indirect-DMA gather path (128 slots, S-scaled bf16 rows) handles
bucket-capacity overflow exactly.  Host-side prep is index routing, dtype
casts and relayout only; all model arithmetic runs on device.
"""
import os, sys

os.environ.setdefault("JAX_PLATFORMS", "cpu,axon")
try:
    import concourse.bass  # noqa: F401
except ImportError:
    sys.path.insert(0, "/opt/trn_rl_repo")

import numpy as np
import concourse.bass as bass
import concourse.tile as tile
import concourse.mybir as mybir
from concourse import bacc
from concourse.bass_utils import run_bass_kernel_spmd

N_CORES = 8
B, S = 256, 512
V = 50000
M1, M2, EMB = 1024, 512, 256
RPC = B // N_CORES           # rows per core after the exchange = 32
NB = 50                      # vocab buckets per core (50*128 = 6400)
NP = NB // 2                 # DoubleRow bucket pairs = 25
VSH = NB * 128               # vocab shard size = 6400
P_B = 256                    # token slots per (bucket, row-half) cell
TPB = 4                      # tiles per bucket = 2 halves x 2 tiles
NT = NB * TPB                # token tiles per core = 200
SPILL = 128                  # overflow token slots per core
WSCALE = 1024.0              # fp8 weight scale S (descaled at h1p copy)

_CACHE = {}


def _build(reps=1, sim=False):
    nc = bacc.Bacc("TRN2", target_bir_lowering=False, debug=False,
                   num_devices=1 if sim else N_CORES)
    f32 = mybir.dt.float32
    bf16 = mybir.dt.bfloat16
    f8 = mybir.dt.float8e4

    w1a = nc.dram_tensor("w1a", [128, NB, M1], f8, kind="ExternalInput")
    w1b = nc.dram_tensor("w1b", [128, NB, M1], f8, kind="ExternalInput")
    w1s = nc.dram_tensor("w1s", [VSH, M1], bf16, kind="ExternalInput")
    # packed f32 consts: rv | rw | b1tt | b2tt  -> [128, NT+NT+8+4]
    NCF = NT + NT + M1 // 128 + M2 // 128
    cfp = nc.dram_tensor("cfp", [128, NCF], f32, kind="ExternalInput")
    msph = nc.dram_tensor("msph", [128, 256], f8, kind="ExternalInput")
    w2t = nc.dram_tensor("w2t", [128, M1 // 128, M2], bf16, kind="ExternalInput")
    w3t = nc.dram_tensor("w3t", [128, M2 // 128, EMB], bf16, kind="ExternalInput")
    b3r = nc.dram_tensor("b3r", [1, EMB], bf16, kind="ExternalInput")
    s4id = nc.dram_tensor("s4id", [128, RPC], bf16, kind="ExternalInput")
    spidx = nc.dram_tensor("spidx", [128, 1], mybir.dt.int32, kind="ExternalInput")
    emb = nc.dram_tensor("emb", [reps * RPC, EMB], f32, kind="ExternalOutput")

    with tile.TileContext(nc) as tc:
        with (
            tc.tile_pool(name="const", bufs=1) as cpool,
            tc.tile_pool(name="etab", bufs=1) as epool,
            tc.tile_pool(name="bowsb", bufs=1) as bpool,
            tc.tile_pool(name="act", bufs=2) as apool,
            tc.tile_pool(name="oh", bufs=16) as ohpool,
            tc.tile_pool(name="pbow", bufs=2, space="PSUM") as pbow_pool,
            tc.tile_pool(name="ph1", bufs=1, space="PSUM") as ph1_pool,
            tc.tile_pool(name="ptail", bufs=1, space="PSUM") as ptail_pool,
            tc.tile_pool(name="dram", bufs=2, space="DRAM") as dpool,
        ):
            # ---- constants; small/early-needed tensors first so the
            # bucket pipeline can start ASAP ----
            spidx_t = cpool.tile([128, 1], mybir.dt.int32)
            nc.sync.dma_start(spidx_t[:], spidx[:])
            cf_t = cpool.tile([128, NCF], f32)
            nc.sync.dma_start(cf_t[:], cfp[:])
            rv_t = cf_t[:, 0:NT]
            rw_t = cf_t[:, NT:2 * NT]
            b1_t = cf_t[:, 2 * NT:2 * NT + M1 // 128]
            b2_t = cf_t[:, 2 * NT + M1 // 128:NCF]
            msp = cpool.tile([128, 256], f8)
            nc.sync.dma_start(msp[:], msph[:])
            s4_t = cpool.tile([128, RPC], bf16)
            nc.sync.dma_start(s4_t[:], s4id[:])
            b3_t = cpool.tile([1, EMB], bf16)
            nc.sync.dma_start(b3_t[:], b3r[:])
            iota_i = cpool.tile([128, 128], mybir.dt.int32)
            nc.gpsimd.iota(iota_i[:], pattern=[[1, 128]], base=0, channel_multiplier=0)
            iotaR = cpool.tile([128, 128], bf16)
            nc.vector.tensor_copy(iotaR[:], iota_i[:])
            ones1 = cpool.tile([1, RPC], bf16)
            nc.gpsimd.memset(ones1[:], 1.0)
            gsp = cpool.tile([128, M1], bf16)
            nc.gpsimd.indirect_dma_start(
                out=gsp[:], out_offset=None, in_=w1s[:],
                in_offset=bass.IndirectOffsetOnAxis(ap=spidx_t[:, 0:1], axis=0),
            )
            w2_t = cpool.tile([128, M1 // 128, M2], bf16)
            nc.sync.dma_start(w2_t[:], w2t[:])
            w3_t = cpool.tile([128, M2 // 128, EMB], bf16)
            nc.sync.dma_start(w3_t[:], w3t[:])

            for _rep in range(reps):
                _body(nc, tc, epool, bpool, apool, ohpool,
                      pbow_pool, ph1_pool, ptail_pool, dpool,
                      gsp, msp, w2_t, w3_t, b1_t, b2_t, b3_t,
                      s4_t, ones1, rv_t, rw_t, iotaR,
                      emb[_rep * RPC:(_rep + 1) * RPC, :], w1a, w1b, sim)

    nc.compile()
    return nc


def _body(nc, tc, epool, bpool, apool, ohpool,
          pbow_pool, ph1_pool, ptail_pool, dpool,
          gsp, msp, w2_t, w3_t, b1_t, b2_t, b3_t,
          s4_t, ones1, rv_t, rw_t, iotaR, emb, w1a, w1b, sim=False):
    f32 = mybir.dt.float32
    bf16 = mybir.dt.bfloat16
    f8 = mybir.dt.float8e4
    Relu = mybir.ActivationFunctionType.Relu
    Copy = mybir.ActivationFunctionType.Copy
    DR = mybir.MatmulPerfMode.DoubleRow
    eq = mybir.AluOpType.is_equal

    # ---- stream both fp8 weight tables, pair-major so stage-2 pair p has
    # (A[2p], A[2p+1], B[2p], B[2p+1]) as early as possible ----
    etA = epool.tile([128, NB, M1], f8, tag="etA")
    etB = epool.tile([128, NB, M1], f8, tag="etB")
    for p in range(NP):
        for q in (2 * p, 2 * p + 1):
            nc.sync.dma_start(etA[:, q:q + 1, :], w1a[:, q:q + 1, :])
        for q in (2 * p, 2 * p + 1):
            nc.sync.dma_start(etB[:, q:q + 1, :], w1b[:, q:q + 1, :])

    # ---- stage 1 (histogram) + stage 2 (DoubleRow fc1), single pass ----
    bowT = bpool.tile([128, NB, 256], f8, tag="bowT")
    ph1 = ph1_pool.tile([128, 2, 2, 512], f32, tag="h1")

    def stage1(q):
        pb = pbow_pool.tile([128, 256], f32, tag="bow")
        for j in range(TPB):
            t = q * TPB + j
            h = j // 2
            rt = ohpool.tile([128, 128], bf16, tag="oh")
            nc.vector.tensor_scalar(rt[:], iotaR[:], rv_t[:, t:t + 1], None, op0=eq)
            rm = ohpool.tile([128, 128], bf16, tag="oh")
            nc.vector.tensor_scalar(rm[:], iotaR[:], rw_t[:, t:t + 1], None, op0=eq)
            nc.tensor.matmul(pb[:, h * 128:(h + 1) * 128], lhsT=rt[:], rhs=rm[:],
                             start=(j % 2 == 0), stop=(j % 2 == 1))
        nc.scalar.activation(bowT[:, q, :], pb[:], Copy)

    def stage2(p):
        for h in range(2):
            for m in range(2):
                nc.tensor.matmul(
                    ph1[:, h, m, :],
                    lhsT=bowT[:, 2 * p:2 * p + 2, h * 128:(h + 1) * 128],
                    rhs=etA[:, 2 * p:2 * p + 2, m * 512:(m + 1) * 512],
                    start=(p == 0), stop=False, perf_mode=DR)
                nc.tensor.matmul(
                    ph1[:, h, m, :],
                    lhsT=bowT[:, 2 * p:2 * p + 2, h * 128:(h + 1) * 128],
                    rhs=etB[:, 2 * p:2 * p + 2, m * 512:(m + 1) * 512],
                    start=False, stop=False, perf_mode=DR)

    for p in range(NP + 1):
        if p < NP:
            stage1(2 * p)
            stage1(2 * p + 1)
        if p >= 1:
            stage2(p - 1)
    # spill contribution (S-scaled bf16 rows, exact) ends the accumulation
    for h in range(2):
        for m in range(2):
            nc.tensor.matmul(ph1[:, h, m, :], lhsT=msp[:, h * 128:(h + 1) * 128],
                             rhs=gsp[:, m * 512:(m + 1) * 512],
                             start=False, stop=(h == 1 and m == 1))
    h1p = apool.tile([128, 2, 2, 512], bf16, tag="h1p")
    for h in range(2):
        for m in range(2):
            nc.scalar.activation(h1p[:, h, m, :], ph1[:, h, m, :], Copy,
                                 scale=1.0 / WSCALE)

    # ---- exchange partial h1: AllToAll (8 chunks of 32 rows) ----
    cc_in = dpool.tile([B, M1], bf16, tag="cc_in")
    cc_out = dpool.tile([B, M1], bf16, tag="cc_out")
    nc.sync.dma_start(
        cc_in[:].rearrange("(h p) (b m) -> p h b m", p=128, b=2), h1p[:])
    if sim:
        nc.sync.dma_start(cc_out[:], cc_in[:])
    else:
        nc.gpsimd.collective_compute(
            "AllToAll", mybir.AluOpType.bypass,
            replica_groups=[list(range(N_CORES))],
            ins=[cc_in[:]], outs=[cc_out[:]],
        )
    cc_sb = apool.tile([128, 2, M1], bf16, tag="ccsb")
    nc.sync.dma_start(
        cc_sb[:], cc_out[:].rearrange("(d q r) m -> (q r) d m", d=2, q=4))

    # ---- sum the 8 partials on TensorE, TRANSPOSED: h1T [feat128, a, rows] ----
    pt1 = ptail_pool.tile([128, M1 // 128, RPC], f32, tag="tail")
    for a in range(M1 // 128):
        for d in range(2):
            nc.tensor.matmul(pt1[:, a, :], lhsT=cc_sb[:, d, a * 128:(a + 1) * 128],
                             rhs=s4_t[:], start=(d == 0), stop=(d == 1))
    h1T = apool.tile([128, M1 // 128, RPC], bf16, tag="h1T")
    for a in range(M1 // 128):
        nc.scalar.activation(h1T[:, a, :], pt1[:, a, :], Relu,
                             bias=b1_t[:, a:a + 1])

    # ---- fc2, output transposed: h2T [feat128, m4, rows] ----
    pt2 = ptail_pool.tile([128, M1 // 128, RPC], f32, tag="tail")
    for m4 in range(M2 // 128):
        for a in range(M1 // 128):
            nc.tensor.matmul(pt2[:, m4, :], lhsT=w2_t[:, a, m4 * 128:(m4 + 1) * 128],
                             rhs=h1T[:, a, :],
                             start=(a == 0), stop=(a == M1 // 128 - 1))
    h2T = apool.tile([128, M2 // 128, RPC], bf16, tag="h2T")
    for m4 in range(M2 // 128):
        nc.scalar.activation(h2T[:, m4, :], pt2[:, m4, :], Relu,
                             bias=b2_t[:, m4:m4 + 1])

    # ---- fc3, row-major output [32, 256] ----
    pt3f = ptail_pool.tile([128, M1 // 128, RPC], f32, tag="tail")
    pt3 = pt3f[0:RPC, 0:EMB // RPC, :]
    for m4 in range(M2 // 128):
        nc.tensor.matmul(pt3, lhsT=h2T[:, m4, :], rhs=w3_t[:, m4, :],
                         start=(m4 == 0), stop=False)
    nc.tensor.matmul(pt3, lhsT=ones1[:], rhs=b3_t[:], start=False, stop=True)
    out_t = apool.tile([RPC, EMB], f32, tag="out")
    nc.scalar.activation(out_t[:], pt3, Relu)
    nc.sync.dma_start(emb[:], out_t[:])


def _prep_inputs(idx, W1, b1, W2, b2, W3, b3):
    """Host-side sharding/layout prep (index routing + dtype/layout only)."""
    import ml_dtypes

    bf16 = ml_dtypes.bfloat16
    f8np = mybir.dt.np(mybir.dt.float8e4)
    idx = np.asarray(idx).astype(np.int64)
    VPAD = N_CORES * VSH
    w1f = np.zeros((VPAD, M1), dtype=np.float32)
    w1f[:V] = np.asarray(W1, dtype=np.float32).T
    w1f *= WSCALE
    w1A = w1f.astype(f8np)
    w1B = (w1f - w1A.astype(np.float32)).astype(f8np)
    w1sc = w1f.astype(bf16)          # S-scaled bf16 rows for the spill gather

    w2t = np.ascontiguousarray(
        np.asarray(W2, dtype=np.float32).T.reshape(M1 // 128, 128, M2)
        .transpose(1, 0, 2)).astype(bf16)
    w3t = np.ascontiguousarray(
        np.asarray(W3, dtype=np.float32).T.reshape(M2 // 128, 128, EMB)
        .transpose(1, 0, 2)).astype(bf16)
    b1tt = np.ascontiguousarray(
        np.asarray(b1, dtype=np.float32).reshape(M1 // 128, 128).T)
    b2tt = np.ascontiguousarray(
        np.asarray(b2, dtype=np.float32).reshape(M2 // 128, 128).T)
    b3r = np.asarray(b3, dtype=np.float32).reshape(1, EMB).astype(bf16)
    s4id = (np.arange(128)[:, None] % RPC == np.arange(RPC)[None, :]).astype(bf16)

    rows = np.repeat(np.arange(B, dtype=np.int64), S)
    vals = idx.reshape(-1)
    core = vals // VSH
    in_maps = []
    for c in range(N_CORES):
        sel = core == c
        v = vals[sel] - c * VSH
        r = rows[sel]
        q = v // 128
        rl = v % 128
        order = np.argsort(q, kind="stable")
        q, rl, r, v = q[order], rl[order], r[order], v[order]

        rv_arr = np.full((NT * 128,), 200, dtype=np.int64)
        rw_arr = np.full((NT * 128,), 300, dtype=np.int64)
        sp_idx = np.zeros((SPILL,), dtype=np.int32)
        sp_row = np.full((SPILL,), 300, dtype=np.int64)
        n_spill = 0
        for qq in range(NB):
            for hh in range(2):
                m = (q == qq) & ((r // 128) == hh)
                nq = int(m.sum())
                take = min(nq, P_B)
                base = (qq * 4 + hh * 2) * 128
                rv_arr[base:base + take] = rl[m][:take]
                rw_arr[base:base + take] = r[m][:take] % 128
                if nq > take:
                    ov = nq - take
                    assert n_spill + ov <= SPILL, "spill capacity exceeded"
                    sp_idx[n_spill:n_spill + ov] = v[m][take:]
                    sp_row[n_spill:n_spill + ov] = r[m][take:]
                    n_spill += ov
        rv_til = rv_arr.reshape(NT, 128).T        # [128, NT]
        rw_til = rw_arr.reshape(NT, 128).T
        mspa = (sp_row[:, None] == np.arange(256)[None, :]).astype(f8np)

        w1Ac = w1A[c * VSH:(c + 1) * VSH]                     # [6400, 1024]
        w1Bc = w1B[c * VSH:(c + 1) * VSH]
        w1atl = np.ascontiguousarray(
            w1Ac.reshape(NB, 128, M1).transpose(1, 0, 2))     # [128, 50, 1024]
        w1btl = np.ascontiguousarray(
            w1Bc.reshape(NB, 128, M1).transpose(1, 0, 2))

        in_maps.append({
            "w1a": w1atl,
            "w1b": w1btl,
            "w1s": np.ascontiguousarray(w1sc[c * VSH:(c + 1) * VSH]),
            "rv": np.ascontiguousarray(rv_til.astype(np.float32)),
            "rw": np.ascontiguousarray(rw_til.astype(np.float32)),
            "msph": mspa,
            "w2t": w2t, "w3t": w3t,
            "b1tt": b1tt, "b2tt": b2tt, "b3r": b3r, "s4id": s4id,
            "spidx": sp_idx.reshape(128, 1),
        })
    return in_maps


def kernel(idx, W1, b1, W2, b2, W3, b3):
    if "nc" not in _CACHE:
        _CACHE["nc"] = _build()
    nc = _CACHE["nc"]
    in_maps = _prep_inputs(idx, W1, b1, W2, b2, W3, b3)
    try:
        res = run_bass_kernel_spmd(nc, in_maps, list(range(N_CORES)))
    except Exception:
        res = run_bass_kernel_spmd(nc, in_maps, list(range(N_CORES)))
    return np.concatenate([res.results[c]["emb"] for c in range(N_CORES)], axis=0)
